# revision 34
# baseline (speedup 1.0000x reference)
"""AMSoftmax (norm-free branch) Trainium2 kernel, 8 NeuronCores.

Reference computes, for input x [B,D], label [B], weight [C,D], scalars s,m:
    norm   = ||x||_2 per row                       [B,1]
    cosine = (x/max(norm,eps)) @ (w/max(||w||,eps)).T   [B,C]
    logits = norm * (cosine - m*onehot(label))     [B,C]
    returns (logits, cosine)

Key identity: norm * cosine == x @ w_hat.T exactly (norm/max(norm,eps) == 1
for any nonzero x), so per output element:
    raw    = x @ w_hat.T          (PSUM, f32)
    cosine = raw * (1/norm)       (per-row scale, ACT)
    logits = raw - norm*m*onehot  (DVE sub against a sparse mask)

Sharding: 2-way over batch x 4-way over classes (8 cores, no collectives;
outputs are disjoint tiles concatenated on host). Per core: x [2048,512],
w_hat [2000,512], outputs [2048,2000] each, stored as bf16 (outputs are
upcast to f32 on host; rel-err budget 2e-2 >> bf16 rounding).

Margin mask is built per row tile by a GPSIMD local_scatter (one indexed
write per partition) instead of a full iota-compare pass on DVE.

Matmuls use 1000-wide bf16 moving operands (2 PSUM banks), 4 accumulating
matmuls per (row tile, column half) -> 128 matmuls total per core.
"""

import os
import sys

sys.path.insert(0, "/opt/trn_rl_repo")

import numpy as np

B, D, C = 4096, 512, 8000
NB, NCL = 2, 4  # batch x class core grid
BL, CL = B // NB, C // NCL  # 2048, 2000 per core
RT = BL // 128  # 16 row tiles
KC = D // 128  # 4 contraction chunks
CW = 500  # matmul free-dim chunk (PSUM bank holds 512 f32)
HB = 2  # 500-col chunks per column half
HW_ = CW * HB  # 1000 columns per half
NH = CL // HW_  # 2 column halves per row tile

COMPUTE = os.environ.get("AMS_DTYPE", "bf16")  # "bf16" or "f32r"
OUT_BF16 = os.environ.get("AMS_OUT", "bf16") == "bf16"  # store outputs bf16
WARMUP_MM = int(os.environ.get("AMS_WARMUP", "12"))

_CACHE = {}


def _build():
    import concourse.mybir as mybir
    import concourse.tile as tile
    from concourse import bacc, library_config
    from concourse.masks import make_identity

    f32 = mybir.dt.float32
    i16 = mybir.dt.int16
    bf16 = mybir.dt.bfloat16
    cdt = bf16 if COMPUTE == "bf16" else mybir.dt.float32r
    odt = bf16 if OUT_BF16 else f32  # DRAM output dtype

    nc = bacc.Bacc()
    x_ext = nc.declare_dram_parameter("x", [BL, D], f32, isOutput=False)
    w_ext = nc.declare_dram_parameter("w", [CL, D], f32, isOutput=False)
    labx_ext = nc.declare_dram_parameter("labx", [128, 2 * RT], i16, isOutput=False)
    m_ext = nc.declare_dram_parameter("mvec", [128, 1], f32, isOutput=False)
    logits_ext = nc.declare_dram_parameter("logits", [BL, CL], odt, isOutput=True)
    cosine_ext = nc.declare_dram_parameter("cosine", [BL, CL], odt, isOutput=True)

    WT = (CL + 127) // 128  # 16 w row tiles (last one 80 partitions)

    with tile.TileContext(nc) as tc:
        with (
            tc.tile_pool(name="persist", bufs=1) as persist,
            tc.tile_pool(name="sq", bufs=4) as sq_pool,
            tc.tile_pool(name="xbf", bufs=3) as xbf_pool,
            tc.tile_pool(name="psT", bufs=2, space="PSUM") as psT_pool,
            tc.tile_pool(name="psM", bufs=3, space="PSUM") as psM_pool,
            tc.tile_pool(name="outb", bufs=6) as out_pool,
            tc.tile_pool(name="mm", bufs=5) as mm_pool,
        ):
            nc.gpsimd.load_library(library_config.local_scatter)

            identity = persist.tile([128, 128], cdt)
            make_identity(nc, identity)

            labx_sb = persist.tile([128, 2 * RT], i16)
            nc.scalar.dma_start(labx_sb[:], labx_ext[:])
            m_sb = persist.tile([128, 1], f32)
            nc.scalar.dma_start(m_sb[:], m_ext[:])

            w_in = persist.tile([128, WT, D], f32)
            nc.vector.memset(w_in[64:, WT - 1, :], 0.0)
            x_in = persist.tile([128, RT, D], f32)
            w_bf = persist.tile([128, WT, D], cdt)  # normalized W, compute dtype
            wts = []
            for k in range(KC):
                wt_k = persist.tile([128, CL], cdt, tag=f"wt{k}")
                wts.append(wt_k)
            xts = []
            for t in range(RT):
                xt_t = persist.tile([128, KC, 128], cdt, tag=f"xt{t}")
                xts.append(xt_t)

            xss = persist.tile([128, RT], f32)
            xnorm = persist.tile([128, RT], f32)
            inv_xnorm = persist.tile([128, RT], f32)
            norm_m = persist.tile([128, RT], f32)
            normm2 = persist.tile([128, 2 * RT], bf16)  # scatter data pairs
            wss = persist.tile([128, WT], f32)
            inv_wnorm = persist.tile([128, WT], f32)
            # last W tile covers only 80 rows; pad so batched norm ops on
            # full partitions read defined data
            nc.vector.memset(wss[:], 1.0)
            # preload both ACT function tables (Square loads with the first
            # real op; Sqrt/Copy otherwise reload 1.3us mid-chain)
            dumm = persist.tile([128, 1], f32)
            nc.scalar.sqrt(dumm[:], wss[:, :1])
            nc.scalar.copy(dumm[:], wss[:, :1])

            # ---- batched input DMAs. labx/m/x tiles 0-1 on the ACT HWDGE
            # ring (they gate the margin-mask chain); W as 8 half-MB pairs on
            # the sync ring so the per-pair prep chain starts on the first
            # 0.5 MB instead of the first 1 MB. Remaining x tiles dispatch
            # from the ACT ring at spread-out emission points. ----
            nc.scalar.dma_start(
                x_in[:, 0:2, :],
                x_ext[0:256, :].rearrange("(a p) d -> p a d", p=128),
            )
            for pr in range(7):
                nc.sync.dma_start(
                    w_in[:, 2 * pr : 2 * pr + 2, :],
                    w_ext[256 * pr : 256 * (pr + 1), :].rearrange(
                        "(a p) d -> p a d", p=128
                    ),
                )
            nc.sync.dma_start(w_in[:, 14, :], w_ext[1792:1920, :])
            nc.sync.dma_start(w_in[:80, 15, :], w_ext[1920:2000, :])

            def x_load(t0, t1):
                nc.scalar.dma_start(
                    x_in[:, t0:t1, :],
                    x_ext[128 * t0 : 128 * t1, :].rearrange(
                        "(a p) d -> p a d", p=128
                    ),
                )

            # ---- W prep: per-pair chain (tiles 2pr, 2pr+1), ACT and DVE
            # working one tile each in parallel ----
            def w_pair(pr):
                a, b = 2 * pr, 2 * pr + 1
                pa = min(128, CL - a * 128)
                pb = min(128, CL - b * 128)
                sqa = sq_pool.tile([128, D], f32, tag="sq")
                sqb = sq_pool.tile([128, D], f32, tag="sq")
                nc.scalar.activation(
                    sqa[:],
                    w_in[:, a, :],
                    mybir.ActivationFunctionType.Square,
                    accum_out=wss[:, a : a + 1],
                )
                nc.vector.tensor_mul(sqb[:], w_in[:, b, :], w_in[:, b, :])
                nc.vector.reduce_sum(
                    wss[:, b : b + 1], sqb[:], axis=mybir.AxisListType.X
                )
                cs = slice(a, b + 1)
                nc.scalar.sqrt(wss[:, cs], wss[:, cs])
                nc.vector.tensor_scalar_max(wss[:, cs], wss[:, cs], 1e-12)
                nc.vector.reciprocal(inv_wnorm[:, cs], wss[:, cs])
                nc.scalar.mul(
                    w_bf[:pa, a, :], w_in[:pa, a, :], inv_wnorm[:pa, a : a + 1]
                )
                nc.vector.tensor_scalar_mul(
                    w_bf[:pb, b, :], w_in[:pb, b, :], inv_wnorm[:pb, b : b + 1]
                )
                for k in range(KC):
                    ps = psT_pool.tile([128, 2, 128], cdt, tag="psT")
                    nc.tensor.transpose(
                        ps[:, 0, :pa],
                        w_bf[:pa, a, k * 128 : (k + 1) * 128],
                        identity[:pa, :pa],
                    )
                    nc.tensor.transpose(
                        ps[:, 1, :pb],
                        w_bf[:pb, b, k * 128 : (k + 1) * 128],
                        identity[:pb, :pb],
                    )
                    eng = nc.vector.tensor_copy if pr % 2 == 0 else nc.scalar.copy
                    if pr < 7:
                        eng(wts[k][:, 256 * pr : 256 * (pr + 1)], ps[:])
                    else:
                        eng(wts[k][:, 1792:1920], ps[:, 0, :])
                        eng(wts[k][:, 1920:2000], ps[:, 1, :80])

            # ---- X prep: row sumsq (ACT/DVE), cast, transpose (PE), drain ----
            def x_sq(t, force_act=False):
                sq = sq_pool.tile([128, D], f32, tag="sq")
                if force_act:
                    nc.scalar.activation(
                        sq[:],
                        x_in[:, t, :],
                        mybir.ActivationFunctionType.Square,
                        accum_out=xss[:, t : t + 1],
                    )
                elif t % 2 == 0:
                    nc.scalar.activation(
                        sq[:],
                        x_in[:, t, :],
                        mybir.ActivationFunctionType.Square,
                        accum_out=xss[:, t : t + 1],
                    )
                else:
                    nc.vector.tensor_mul(sq[:], x_in[:, t, :], x_in[:, t, :])
                    nc.vector.reduce_sum(
                        xss[:, t : t + 1], sq[:], axis=mybir.AxisListType.X
                    )

            def x_tr(t):
                # bf16 cast on GPSIMD: keeps it out of the ACT/DVE epilogue
                # backlog so the PE transposes are never input-starved
                xb = xbf_pool.tile([128, D], cdt, tag="xb")
                nc.gpsimd.tensor_copy(xb[:], x_in[:, t, :])
                ps = psT_pool.tile([128, KC, 128], cdt, tag="psT")
                for k in range(KC):
                    nc.tensor.transpose(
                        ps[:, k, :],
                        xb[:, k * 128 : (k + 1) * 128],
                        identity[:],
                    )
                if t % 2 == 0:
                    nc.scalar.copy(xts[t][:], ps[:])
                else:
                    nc.vector.tensor_copy(xts[t][:], ps[:])

            def x_prep(t):
                x_sq(t)
                x_tr(t)

            def x_norms(g):
                cs = slice(2 * g, 2 * g + 2)
                nc.scalar.sqrt(xnorm[:, cs], xss[:, cs])
                nc.vector.tensor_scalar_max(xnorm[:, cs], xnorm[:, cs], 1e-12)
                nc.vector.reciprocal(inv_xnorm[:, cs], xnorm[:, cs])
                nc.vector.tensor_mul(
                    norm_m[:, cs], xnorm[:, cs], m_sb.broadcast_to([128, 2])
                )

            def normm_pair(t):
                # bf16 [v, v] pair for the scatter data operand
                nc.gpsimd.tensor_copy(
                    normm2[:, 2 * t : 2 * t + 2],
                    norm_m[:, t : t + 1].broadcast_to([128, 2]),
                )

            # ---- main loop body ----
            masks = {}

            def mask(t):
                mmt = mm_pool.tile([128, CL], bf16, tag="mm")
                nc.gpsimd.local_scatter(
                    mmt[:],
                    normm2[:, 2 * t : 2 * t + 2],
                    labx_sb[:, 2 * t : 2 * t + 2],
                    channels=128,
                    num_elems=CL,
                    num_idxs=2,
                )
                masks[t] = mmt

            def main_h(t, h):
                ps = psM_pool.tile([128, HB, 512], f32, tag="psM")
                for k in range(KC):
                    for cc in range(HB):
                        c0 = h * HW_ + cc * CW
                        nc.tensor.matmul(
                            ps[:, cc, :CW],
                            xts[t][:, k, :],
                            wts[k][:, c0 : c0 + CW],
                            start=(k == 0),
                            stop=(k == KC - 1),
                        )
                ps3 = ps[:, :, :CW]
                cos_h = out_pool.tile([128, HW_], odt, tag="cos")
                log_h = out_pool.tile([128, HW_], odt, tag="log")
                mmt = masks[t]
                nc.scalar.activation(
                    cos_h[:].rearrange("p (a b) -> p a b", a=HB),
                    ps3,
                    mybir.ActivationFunctionType.Copy,
                    scale=inv_xnorm[:, t : t + 1],
                )
                nc.vector.tensor_sub(
                    log_h[:].rearrange("p (a b) -> p a b", a=HB),
                    ps3,
                    mmt[:, h * HW_ : (h + 1) * HW_].rearrange(
                        "p (a b) -> p a b", a=HB
                    ),
                )
                if h == NH - 1:
                    masks.pop(t)
                r0, r1 = t * 128, (t + 1) * 128
                c0, c1 = h * HW_, (h + 1) * HW_
                nc.sync.dma_start(cosine_ext[r0:r1, c0:c1], cos_h[:])
                nc.sync.dma_start(logits_ext[r0:r1, c0:c1], log_h[:])

            # ---- emission. Head ordering: the margin-mask chain for tiles
            # 0/1 (x sumsq -> norms -> scatter) runs on ACT/DVE/GPSIMD before
            # the W-prep chain floods those queues, and the W transpose burst
            # is split so the first matmuls start after only column half 0 of
            # W is transposed. ----
            x_sq(0, force_act=True)
            x_sq(1, force_act=True)
            x_load(2, 4)
            for pr in range(4):
                w_pair(pr)
            x_norms(0)
            normm_pair(0)
            normm_pair(1)
            mask(0)
            x_tr(0)
            x_tr(1)
            mask(1)
            main_h(0, 0)
            x_prep(2)
            main_h(1, 0)
            x_load(4, 8)
            for pr in range(4, 8):
                w_pair(pr)
            x_prep(3)
            x_norms(1)
            normm_pair(2)
            normm_pair(3)
            mask(2)
            main_h(2, 0)
            x_load(8, 12)
            main_h(0, 1)
            mask(3)
            main_h(1, 1)
            main_h(2, 1)
            x_prep(4)
            x_load(12, 16)
            x_prep(5)
            x_norms(2)
            normm_pair(4)
            normm_pair(5)
            mask(4)
            main_h(3, 0)
            main_h(3, 1)
            for t in range(4, RT):
                nt = t + 2
                if nt < RT:
                    x_prep(nt)
                    if nt % 2 == 1:
                        x_norms(nt // 2)
                        normm_pair(nt - 1)
                        normm_pair(nt)
                if t + 1 < RT:
                    mask(t + 1)
                main_h(t, 0)
                main_h(t, 1)

    nc.finalize()
    return nc


def _in_maps(x, w, lab, mval):
    maps = []
    lab = np.asarray(lab).astype(np.int64)
    for ci in range(8):
        bi, cj = ci // NCL, ci % NCL
        b0, c0 = bi * BL, cj * CL
        ll = (lab[b0 : b0 + BL] - c0).reshape(RT, 128).T  # [128, RT]
        valid = (ll >= 0) & (ll < CL)
        labx = np.full((128, 2 * RT), -2, dtype=np.int16)
        labx[:, 0::2] = np.where(valid, ll, -1).astype(np.int16)
        maps.append(
            {
                "x": x[b0 : b0 + BL],
                "w": w[c0 : c0 + CL],
                "labx": np.ascontiguousarray(labx),
                "mvec": np.full((128, 1), mval, dtype=np.float32),
            }
        )
    return maps


def kernel(input, label, weight, s, m):
    from concourse.bass_utils import run_bass_kernel_spmd

    if "nc" not in _CACHE:
        _CACHE["nc"] = _build()
    nc = _CACHE["nc"]

    x = np.ascontiguousarray(np.asarray(input, dtype=np.float32))
    w = np.ascontiguousarray(np.asarray(weight, dtype=np.float32))
    lab = np.asarray(label)
    mval = float(np.asarray(m))

    res = run_bass_kernel_spmd(nc, _in_maps(x, w, lab, mval), core_ids=list(range(8)))

    logits = np.empty((B, C), dtype=np.float32)
    cosine = np.empty((B, C), dtype=np.float32)
    for ci in range(8):
        bi, cj = ci // NCL, ci % NCL
        b0, c0 = bi * BL, cj * CL
        logits[b0 : b0 + BL, c0 : c0 + CL] = np.asarray(
            res.results[ci]["logits"], dtype=np.float32
        )
        cosine[b0 : b0 + BL, c0 : c0 + CL] = np.asarray(
            res.results[ci]["cosine"], dtype=np.float32
        )
    return logits, cosine
